# revision 2
# baseline (speedup 1.0000x reference)
"""Guided filter (r=40, eps=1e-3) on 8 Trainium2 NeuronCores.

Sharding: pure data-parallel over the batch dim (8 batches -> 8 cores).
Each core processes 3 channel-images of 512x512.

Algorithm per image:
  box2d(x) done as two banded matmuls on the TensorEngine (version "A":
  the image chunk is the stationary operand, the 0/1 band matrix the
  moving operand; contraction runs over the partition dim so each pass
  both box-filters one axis and transposes the layout).
  - V-pass band columns carry 2^round(log2(1/n_h)) (exact in bf16); the
    per-row residual rho_h is applied later as a per-partition scalar.
  - W-pass band columns carry bf16(1/n_w).
  - eps is added to the II box output via a rank-1 (K=1) matmul.
  Elementwise stage on VectorE/ScalarE consuming PSUM directly.
"""

import os
import sys
import numpy as np
import ml_dtypes
from contextlib import ExitStack

sys.path.insert(0, "/opt/trn_rl_repo")

import concourse.bass as bass
import concourse.tile as tile
from concourse import bacc, mybir
from concourse.bass_utils import run_bass_kernel_spmd

F32 = mybir.dt.float32
BF16 = mybir.dt.bfloat16
ALU = mybir.AluOpType

R = 40
EPS = 1e-3
HW_ = 512
NB = 4  # 128-row blocks per axis
CH = 3  # channels per batch
P = 128
NCORES = 8


def _band_range(c):
    n0 = max(0, P * c - R)
    n1 = min(HW_, P * c + P + R)
    return n0, n1


def make_consts():
    idx = np.arange(HW_)
    n1d = (np.minimum(idx + R, HW_ - 1) - np.maximum(idx - R, 0) + 1).astype(np.float64)
    inv_n = 1.0 / n1d
    E = np.round(np.log2(inv_n))
    po2 = 2.0 ** E                      # exact in bf16
    rho = (inv_n * 2.0 ** (-E)).astype(np.float32)   # residual, ~[0.7, 1.42]

    mask = (np.abs(idx[:, None] - idx[None, :]) <= R)
    bandV = (mask * po2[None, :]).astype(ml_dtypes.bfloat16)
    bandW = (mask * inv_n[None, :]).astype(ml_dtypes.bfloat16)
    # [512k, 512n] -> [128 kp, 4*512 (kb, n)]
    bandV = np.ascontiguousarray(
        bandV.reshape(NB, P, HW_).transpose(1, 0, 2).reshape(P, NB * HW_))
    bandW = np.ascontiguousarray(
        bandW.reshape(NB, P, HW_).transpose(1, 0, 2).reshape(P, NB * HW_))

    rho_t = np.ascontiguousarray(rho.reshape(NB, P).T)          # [128, 4]
    eps2e = (EPS / rho).astype(ml_dtypes.bfloat16).reshape(1, HW_)  # [1, 512]
    ones = np.ones((1, HW_), dtype=ml_dtypes.bfloat16)
    return {"bandV": bandV, "bandW": bandW, "rho": rho_t,
            "eps2e": eps2e, "ones": ones}


def _img_view(dram_ap, c):
    # [3, 512, 512] DRAM tensor -> channel c as [128 hp, 4 hb, 512 w]
    return dram_ap[c].rearrange("(hb hp) w -> hp hb w", hp=P)


def _sb3(t):
    # [128, 2048] SBUF tile AP -> [128, 4, 512]
    return t[:].rearrange("p (hb w) -> p hb w", w=HW_)


def build_model():
    nc = bacc.Bacc("TRN2", target_bir_lowering=False, debug=False,
                   num_devices=NCORES)
    I_d = nc.dram_tensor("I", [CH, HW_, HW_], F32, kind="ExternalInput").ap()
    p_d = nc.dram_tensor("p", [CH, HW_, HW_], F32, kind="ExternalInput").ap()
    bandV_d = nc.dram_tensor("bandV", [P, NB * HW_], BF16, kind="ExternalInput").ap()
    bandW_d = nc.dram_tensor("bandW", [P, NB * HW_], BF16, kind="ExternalInput").ap()
    rho_d = nc.dram_tensor("rho", [P, NB], F32, kind="ExternalInput").ap()
    eps2e_d = nc.dram_tensor("eps2e", [1, HW_], BF16, kind="ExternalInput").ap()
    ones_d = nc.dram_tensor("ones", [1, HW_], BF16, kind="ExternalInput").ap()
    out_d = nc.dram_tensor("out", [CH, HW_, HW_], F32, kind="ExternalOutput").ap()

    with tile.TileContext(nc) as tc:
        with ExitStack() as ctx:
            build_kernel(ctx, tc, I_d, p_d, out_d,
                         bandV_d, bandW_d, rho_d, eps2e_d, ones_d)
    nc.compile()
    return nc


def build_kernel(ctx, tc, I_d, p_d, out_d, bandV_d, bandW_d, rho_d,
                 eps2e_d, ones_d):
    nc = tc.nc
    FW = NB * HW_  # 2048

    consts = ctx.enter_context(tc.tile_pool(name="consts", bufs=1))
    bandV = consts.tile_from(bandV_d)
    bandW = consts.tile_from(bandW_d)
    rho = consts.tile_from(rho_d)
    eps2e = consts.tile_from(eps2e_d)
    ones = consts.tile_from(ones_d)

    # image-grain pools (double-buffered across the 3 channels)
    pIf = ctx.enter_context(tc.tile_pool(name="If", bufs=2))
    pPf = ctx.enter_context(tc.tile_pool(name="Pf", bufs=2))
    pBf = ctx.enter_context(tc.tile_pool(name="ibf", bufs=1))
    pY = ctx.enter_context(tc.tile_pool(name="ymid", bufs=2))
    pM = ctx.enter_context(tc.tile_pool(name="means", bufs=1))
    pAB = ctx.enter_context(tc.tile_pool(name="ab", bufs=2))
    pOut = ctx.enter_context(tc.tile_pool(name="outp", bufs=2))
    pT = ctx.enter_context(tc.tile_pool(name="tmps", bufs=2))
    pV = ctx.enter_context(tc.tile_pool(name="psv", bufs=2, space="PSUM"))
    pQ = ctx.enter_context(tc.tile_pool(name="psq", bufs=1, space="PSUM"))
    pRR = ctx.enter_context(tc.tile_pool(name="psr", bufs=1, space="PSUM"))

    def vpass(src_bf, band, psum_pool, dst_bf, copy_eng="act"):
        """One banded pass: src [h|w] bf16 -> dst [w|h] bf16 (box over
        partition axis + transpose). 16 MMs + 4 PSUM->SBUF copies."""
        for i in range(NB):
            ps = psum_pool.tile([P, HW_], F32, tag="ps")
            for j in range(NB):
                n0, n1 = _band_range(j)
                nc.tensor.matmul(
                    ps[:, n0:n1],
                    lhsT=src_bf[:, j * HW_ + i * P: j * HW_ + i * P + P],
                    rhs=band[:, j * HW_ + n0: j * HW_ + n1],
                    start=(j == 0), stop=(j == NB - 1))
            nc.any.tensor_copy(dst_bf[:, i * HW_:(i + 1) * HW_], ps[:])

    def wpass_mm(src_bf, band, q_tile, j, add_eps=False):
        """W-direction banded MMs for output h-chunk j into q_tile."""
        for i in range(NB):
            m0, m1 = _band_range(i)
            last = (i == NB - 1) and not add_eps
            nc.tensor.matmul(
                q_tile[:, m0:m1],
                lhsT=src_bf[:, i * HW_ + j * P: i * HW_ + j * P + P],
                rhs=band[:, i * HW_ + m0: i * HW_ + m1],
                start=(i == 0), stop=last)
        if add_eps:
            nc.tensor.matmul(
                q_tile[:, :],
                lhsT=eps2e[:1, j * P:(j + 1) * P],
                rhs=ones[:1, :],
                start=False, stop=True)

    for c in range(CH):
        I_f = pIf.tile([P, FW], F32, tag="If")
        p_f = pPf.tile([P, FW], F32, tag="pf")
        nc.sync.dma_start(_sb3(I_f), _img_view(I_d, c))
        nc.sync.dma_start(_sb3(p_f), _img_view(p_d, c))

        I_bf = pBf.tile([P, FW], BF16, tag="Ibf")
        p_bf = pBf.tile([P, FW], BF16, tag="pbf")
        Ip_bf = pBf.tile([P, FW], BF16, tag="Ipbf")
        II_bf = pBf.tile([P, FW], BF16, tag="IIbf")
        nc.scalar.copy(I_bf[:], I_f[:])
        nc.scalar.copy(p_bf[:], p_f[:])
        nc.vector.tensor_mul(Ip_bf[:], I_bf[:], p_bf[:])
        nc.scalar.square(II_bf[:], I_f[:])

        # stage 1: V-pass for the four tensors -> [w|h] bf16 mids
        yI = pY.tile([P, FW], BF16, tag="yI")
        yp = pY.tile([P, FW], BF16, tag="yp")
        yIp = pY.tile([P, FW], BF16, tag="yIp")
        yII = pY.tile([P, FW], BF16, tag="yII")
        vpass(I_bf, bandV, pV, yI, "act")
        vpass(p_bf, bandV, pV, yp, "act")
        vpass(Ip_bf, bandV, pV, yIp, "act")
        vpass(II_bf, bandV, pV, yII, "dve")

        # stage 2: W-pass per h-chunk j + elementwise -> a, b (bf16)
        mI = pM.tile([P, FW], F32, tag="mI")
        mI_bf = pM.tile([P, FW], BF16, tag="mIbf")
        mp = pM.tile([P, FW], F32, tag="mp")
        mp_bf = pM.tile([P, FW], BF16, tag="mpbf")
        a_bf = pAB.tile([P, FW], BF16, tag="abf")
        b_bf = pAB.tile([P, FW], BF16, tag="bbf")
        for j in range(NB):
            qI = pQ.tile([P, HW_], F32, tag="qI")
            qp = pQ.tile([P, HW_], F32, tag="qp")
            qIp = pQ.tile([P, HW_], F32, tag="qIp")
            qII = pQ.tile([P, HW_], F32, tag="qII")
            wpass_mm(yI, bandW, qI, j)
            wpass_mm(yp, bandW, qp, j)
            wpass_mm(yIp, bandW, qIp, j)
            wpass_mm(yII, bandW, qII, j, add_eps=True)

            s = rho[:, j:j + 1]
            sl = slice(j * HW_, (j + 1) * HW_)
            mIj = mI[:, sl]
            mpj = mp[:, sl]
            nc.scalar.mul(mIj, qI[:], s)          # mean_I
            nc.scalar.mul(mpj, qp[:], s)          # mean_p
            nc.vector.tensor_copy(mI_bf[:, sl], mIj)
            nc.vector.tensor_copy(mp_bf[:, sl], mpj)
            u = pT.tile([P, HW_], F32, tag="u")
            cov = pT.tile([P, HW_], F32, tag="cov")
            v = pT.tile([P, HW_], F32, tag="v")
            den = pT.tile([P, HW_], F32, tag="den")
            rcp = pT.tile([P, HW_], F32, tag="rcp")
            tt = pT.tile([P, HW_], BF16, tag="tt")
            nc.vector.tensor_mul(u[:], mIj, mpj)
            nc.vector.scalar_tensor_tensor(
                cov[:], qIp[:], s, u[:], op0=ALU.mult, op1=ALU.subtract)
            nc.scalar.square(v[:], mIj)
            nc.vector.scalar_tensor_tensor(
                den[:], qII[:], s, v[:], op0=ALU.mult, op1=ALU.subtract)
            nc.vector.reciprocal_approx_fast(rcp[:], den[:])
            nc.vector.tensor_mul(a_bf[:, sl], cov[:], rcp[:])
            nc.vector.tensor_mul(tt[:], a_bf[:, sl], mI_bf[:, sl])
            nc.vector.tensor_sub(b_bf[:, sl], mp_bf[:, sl], tt[:])

        # stage 3: box2d of a and b, final combine
        ya = pY.tile([P, FW], BF16, tag="ya")
        yb = pY.tile([P, FW], BF16, tag="yb")
        vpass(a_bf, bandV, pV, ya, "dve")
        vpass(b_bf, bandV, pV, yb, "act")

        out_t = pOut.tile([P, FW], F32, tag="out")
        for j in range(NB):
            ra = pRR.tile([P, HW_], F32, tag="ra")
            rb = pRR.tile([P, HW_], F32, tag="rb")
            wpass_mm(ya, bandW, ra, j)
            wpass_mm(yb, bandW, rb, j)
            s = rho[:, j:j + 1]
            sl = slice(j * HW_, (j + 1) * HW_)
            f1 = pT.tile([P, HW_], F32, tag="f1")
            nc.vector.scalar_tensor_tensor(
                f1[:], ra[:], s, I_f[:, sl], op0=ALU.mult, op1=ALU.mult)
            nc.vector.scalar_tensor_tensor(
                out_t[:, sl], rb[:], s, f1[:], op0=ALU.mult, op1=ALU.add)

        nc.sync.dma_start(_img_view(out_d, c), _sb3(out_t))


_NC_CACHE = None
LAST_RESULT = None


def _get_model():
    global _NC_CACHE
    if _NC_CACHE is None:
        _NC_CACHE = build_model()
    return _NC_CACHE


def kernel(I, p, _trace=False):
    global LAST_RESULT
    I = np.asarray(I, dtype=np.float32)
    p = np.asarray(p, dtype=np.float32)
    B = I.shape[0]
    assert I.shape == (B, CH, HW_, HW_), I.shape
    nc = _get_model()
    consts = make_consts()
    in_maps = []
    for k in range(NCORES):
        m = {"I": np.ascontiguousarray(I[k]), "p": np.ascontiguousarray(p[k])}
        m.update(consts)
        in_maps.append(m)
    res = run_bass_kernel_spmd(nc, in_maps, core_ids=list(range(NCORES)),
                               trace=_trace)
    LAST_RESULT = res
    out = np.stack([res.results[k]["out"] for k in range(NCORES)], axis=0)
    return out.astype(np.float32)


if __name__ == "__main__":
    rng = np.random.default_rng(0)
    I = rng.random((8, CH, HW_, HW_), dtype=np.float32)
    p = rng.random((8, CH, HW_, HW_), dtype=np.float32)
    out = kernel(I, p)
    print("out", out.shape, out.dtype, float(out.mean()))

